# revision 1
# baseline (speedup 1.0000x reference)
"""Trainium2 Bass kernel: attention-LSTM decoder (nn_Attention_74698071212133).

Sharding: data-parallel over batch across 8 NeuronCores (64 rows each), weights
replicated.  Each core splits its 64 rows into 2 chunks of 32 that run as
software-pipelined per-chunk pipelines, offset roughly half a step; chunk 1's
LSTM emission is deferred into the next step so its tail overlaps the next
step's chunk-0 attention (the whole recurrence splits cleanly by batch).

Per-core, per chunk, per step (bc=32, T=64, H=512, C=38):
  a  = H_projT + bcast_t(hpT)          (DVE bf16 2x, all adds emitted first)
  th = tanh(a)                         (ACT - the per-step bottleneck engine)
  e  = w_score . th -> psum            (PE, th stationary, one accum group)
  softmax via PE transposes            (split A/B emission)
  ctxT = enc.T @ alpha-blockdiag       (PE [d,b] direct; dense block, one
                                        psum->sbuf copy per chunk)
  gates into one psum [128,(gate,k,b)] (i/f/o rows pre-halved -> single tanh)
  LSTM on doubled states h2=2h, c2=2c  (4 scalar_tensor_tensor ops; consumer
                                        weights w_hh/w_h2h/w_gen pre-halved,
                                        tanh(c) via ACT scale=0.5)
  hp(next) = w_h2h @ h2 -> psum        (PE, per chunk)
h states live in the hs_all ring (k-major [k][s][b]: no copies, no WAR);
probs = one batched matmul over all 26 steps after the scan.
"""

import sys

sys.path.insert(0, "/opt/trn_rl_repo")

import numpy as np
import ml_dtypes

import concourse.bass as bass
import concourse.mybir as mybir
import concourse.tile as tile
from concourse import bacc
from concourse.bass_utils import run_bass_kernel_spmd

BF = ml_dtypes.bfloat16
F32 = mybir.dt.float32
BF16 = mybir.dt.bfloat16
AF = mybir.ActivationFunctionType
ALU = mybir.AluOpType

# Problem constants
B, T, D, H, C, S = 512, 64, 512, 512, 38, 26
NCORES = 8
BCORE = B // NCORES  # 64
NCHUNK = 2
G4 = 4 * H  # 2048
HK = H // 128  # 4 h-tiles
GW = HK * BCORE  # 256: one h-state slot (k-major, then b)
PCOL = (0, 1, 3, 2)  # W gate order i,f,g,o -> psum col block [i | f | o | g]


def _tile128(a):
    """[R, N] with R = r*128 -> [128, r*N] col-block layout (block k = rows 128k..)."""
    r = a.shape[0] // 128
    return np.ascontiguousarray(
        a.reshape(r, 128, a.shape[1]).transpose(1, 0, 2).reshape(128, -1)
    )


def build_nc(steps=S, nchunk=NCHUNK, n_gps_adds=0):
    bc = BCORE // nchunk  # batch per chunk
    bt = bc * T  # flattened (b, t) per chunk, b-major
    nbt = bt // 128  # 128-row bt tiles per chunk

    nc = bacc.Bacc()
    dp = nc.declare_dram_parameter
    # Per-core tensors (pre-tiled on host into [128, cols] SBUF images)
    d_enc = dp("enc_sb", [nchunk, 128, nbt * 512], BF16, isOutput=False)
    d_encT = dp("encT_sb", [nchunk, 128, HK * bt], BF16, isOutput=False)
    d_oh = dp("ohT_sb", [128, steps * BCORE], BF16, isOutput=False)
    # Replicated weights
    d_wi2h = dp("w_i2hT", [128, HK * H], BF16, isOutput=False)
    d_wh2h = dp("w_h2hT", [128, HK * H], BF16, isOutput=False)
    d_wsc = dp("w_scoreT", [128, HK], BF16, isOutput=False)
    d_wctx = dp("w_ctxT", [128, HK * G4], BF16, isOutput=False)
    d_whh = dp("w_hhT", [128, HK * G4], BF16, isOutput=False)
    d_woh = dp("w_ohT", [128, G4], BF16, isOutput=False)
    d_wgen = dp("w_genT", [128, HK * C], BF16, isOutput=False)
    d_bgen = dp("b_gen", [1, C], BF16, isOutput=False)
    d_bh2h = dp("b_h2hT", [128, HK], F32, isOutput=False)
    d_idf = dp("id_f32", [128, 128], F32, isOutput=False)
    d_ones = dp("ones_row", [1, 128], BF16, isOutput=False)
    d_out = dp("probs", [steps * BCORE, C], F32, isOutput=True)  # s-major rows

    with tile.TileContext(nc) as tc:
        with (
            tc.tile_pool(name="consts", bufs=1) as pc,
            tc.tile_pool(name="persist", bufs=1) as pp,
        ):
            # ---- load constants ----
            def cload(dram, shape, dt):
                t_ = pc.tile(list(shape), dt, name=dram.tensor.name + "_sb")
                nc.sync.dma_start(t_[:], dram)
                return t_

            # only what init needs loads first; the 4MB+ of gate weights
            # (w_ctx/w_hh/...) load AFTER encT so they don't starve H_proj
            w_i2h = cload(d_wi2h[:], [128, HK * H], BF16)
            w_sc = cload(d_wsc[:], [128, HK], BF16)
            b_gen = cload(d_bgen[:], [1, C], BF16)
            b_h2h = cload(d_bh2h[:], [128, HK], F32)
            id_f = cload(d_idf[:], [128, 128], F32)
            ones = cload(d_ones[:], [1, 128], BF16)

            # ---- persistent state ----
            # hs_all: ring of h states, slot s+1 = h after step s (slot 0 = 0)
            # hs_all layout: k-major [k][s][b]; slot s+1 = h after step s
            hs_all = pp.tile([128, HK * (steps + 1) * BCORE], BF16, tag="hs_all")
            hsv = hs_all[:].rearrange(
                "p (k s b) -> p k s b", k=HK, s=steps + 1
            )
            cT = pp.tile([128, GW], F32, tag="cT")
            ctxT = pp.tile([128, GW], BF16, tag="ctxT")
            for k in range(HK):
                nc.vector.memset(hsv[:, k, 0, :], 0.0)
            nc.vector.memset(cT[:], 0.0)

            enc_sb, hproj, ad = [], [], []
            for c in range(nchunk):
                e_ = pp.tile([128, nbt * 512], BF16, tag=f"enc{c}")
                enc_sb.append(e_)
                hproj.append(
                    pp.tile([128, HK * bt], BF16, tag=f"hproj{c}", name=f"hproj{c}")
                )
                a_ = pp.tile([128, bc], BF16, tag=f"ad{c}", name=f"ad{c}")
                nc.vector.memset(a_[:], 0.0)
                ad.append(a_)

            # ---- init: H_projT = w_i2h @ encT + b_h2h ----
            with (
                tc.tile_pool(name="encT", bufs=1) as pet,
                tc.tile_pool(name="initps", bufs=4, space="PSUM") as pips,
            ):
                for c in range(nchunk):
                    et = pet.tile([128, HK * bt], BF16, tag=f"encT{c}", name=f"encT{c}")
                    # column-sliced loads: slice n carries ALL k-blocks for
                    # bt-range n, so the n-th group of MMs starts after 1/nq
                    # of the load instead of waiting for the whole chunk
                    nq = bt // 512
                    etv = et[:].rearrange("p (k c2) -> p k c2", k=HK)
                    dv = d_encT[c].rearrange("p (k c2) -> p k c2", k=HK)
                    for n in range(nq):
                        nc.sync.dma_start(
                            etv[:, :, 512 * n : 512 * (n + 1)],
                            dv[:, :, 512 * n : 512 * (n + 1)],
                        )
                    for n in range(nq):
                        for m in range(HK):
                            ps = pips.tile([128, 512], F32, tag="initp")
                            for k in range(HK):
                                nc.tensor.matmul(
                                    ps[:],
                                    w_i2h[:, k * H + 128 * m : k * H + 128 * m + 128],
                                    et[:, k * bt + 512 * n : k * bt + 512 * n + 512],
                                    start=(k == 0),
                                    stop=(k == HK - 1),
                                )
                            nc.scalar.activation(
                                hproj[c][:, m * bt + 512 * n : m * bt + 512 * n + 512],
                                ps[:],
                                AF.Identity,
                                bias=b_h2h[:, m : m + 1],
                            )

            # deferred loads, ordered by first use in step 0:
            # enc c0 (ctx@~17us), gate weights (tanh_all@~20), w_ctx
            # (ctx-gates), w_h2h (php@~24), enc c1 (ctx c1@~25), w_gen (probs)
            for q in range(4):
                w = nbt * 512 // 4
                nc.sync.dma_start(
                    enc_sb[0][:, q * w : (q + 1) * w],
                    d_enc[0, :, q * w : (q + 1) * w],
                )
            w_hh = cload(d_whh[:], [128, HK * G4], BF16)
            w_oh = cload(d_woh[:], [128, G4], BF16)
            ohT = cload(d_oh[:], [128, steps * BCORE], BF16)
            w_ctx = cload(d_wctx[:], [128, HK * G4], BF16)
            w_h2h = cload(d_wh2h[:], [128, HK * H], BF16)
            for c in range(1, nchunk):
                for q in range(4):
                    w = nbt * 512 // 4
                    nc.sync.dma_start(
                        enc_sb[c][:, q * w : (q + 1) * w],
                        d_enc[c, :, q * w : (q + 1) * w],
                    )
            w_gen = cload(d_wgen[:], [128, HK * C], BF16)

            # ---- decode steps ----
            with (
                tc.tile_pool(name="work", bufs=6) as pw,
                tc.tile_pool(name="small", bufs=4) as psm,
                tc.tile_pool(name="ps_mix", bufs=2, space="PSUM") as ps_mix,
                tc.tile_pool(name="ps_tr", bufs=1, space="PSUM") as ps_tr,
                tc.tile_pool(name="ps_ctx", bufs=1, space="PSUM") as ps_ctx,
                tc.tile_pool(name="ps_g", bufs=1, space="PSUM") as ps_g,
            ):
                ntile = (steps * BCORE) // 128  # 13

                def emit_probs_tile(t_):
                    pp_ = ps_mix.tile([128, C], F32, tag="mix", name="pp_")
                    kstride = (steps + 1) * BCORE
                    for k in range(HK):
                        base = k * kstride + (2 * t_ + 1) * BCORE
                        nc.tensor.matmul(
                            pp_[:],
                            hs_all[:, base : base + 128],
                            w_gen[:, k * C : (k + 1) * C],
                            start=(k == 0), stop=False, skip_group_check=True,
                        )
                    nc.tensor.matmul(
                        pp_[:], ones[0:1, :], b_gen[:],
                        start=False, stop=True, skip_group_check=True,
                    )
                    po = psm.tile([128, C], F32, tag="po")
                    nc.vector.tensor_copy(po[:], pp_[:])
                    nc.sync.dma_start(d_out[t_ * 128 : (t_ + 1) * 128, :], po[:])

                php_holder = [None] * NCHUNK
                pending = None
                for s in range(steps):
                    pending = step_body(
                        nc, tc, s, steps, nchunk, bc, bt,
                        pw, psm, ps_mix, ps_tr, ps_ctx, ps_g,
                        enc_sb, hproj, hsv, cT, ctxT, ad,
                        w_h2h, w_sc, w_ctx, w_hh, w_oh,
                        ohT, id_f, php_holder, n_gps_adds, pending,
                    )
                    # probs tile t covers h-slots (2t+1, 2t+2): fully written
                    # once step 2t+2's deferred c1-LSTM has been emitted
                    if s >= 2 and s % 2 == 0:
                        emit_probs_tile((s - 2) // 2)
                pending()  # flush chunk 1's final LSTM

                # ---- final probs tile (12 of 13 were emitted in-scan) ----
                emit_probs_tile(ntile - 1)
    if not nc.is_finalized():
        nc.finalize()
    return nc


def step_body(
    nc, tc, s, steps, nchunk, bc, bt,
    pw, psm, ps_mix, ps_tr, ps_ctx, ps_g,
    enc_sb, hproj, hsv, cT, ctxT, ad,
    w_h2h, w_sc, w_ctx, w_hh, w_oh,
    ohT, id_f, php_holder, n_gps_adds, pending,
):
    nj = bt // 128
    BW = nchunk * bc
    GWc = HK * bc  # per-chunk gates col width

    # -- per-chunk gates psum + hh/oh contributions (PE work during attention)
    pgs = {}

    def gates_hh(c):
        pg = ps_g.tile([128, 4 * GWc], F32, tag=f"g{c}", name=f"pg{c}")
        pgs[c] = pg
        ohsl = ohT[:, s * BW + c * bc : s * BW + (c + 1) * bc]
        for m in range(16):
            gate, k = m // 4, m % 4
            col = pg[:, PCOL[gate] * GWc + k * bc : PCOL[gate] * GWc + (k + 1) * bc]
            for kk in range(HK):
                nc.tensor.matmul(
                    col,
                    w_hh[:, kk * G4 + 128 * m : kk * G4 + 128 * m + 128],
                    hsv[:, kk, s, c * bc : (c + 1) * bc],
                    start=(m == 0 and kk == 0),
                    stop=False,
                    skip_group_check=True,
                )
            nc.tensor.matmul(
                col, w_oh[:, 128 * m : 128 * m + 128], ohsl,
                start=False, stop=False, skip_group_check=True,
            )

    def attention_front(c, interleave=None):
        """hp2 + adds + tanhs + score MMs for chunk c; optionally interleave
        other-chunk ctx work after each k-block (fills PE tanh-wait gaps)."""
        if s > 0:
            php = php_holder[c]
            hp2 = psm.tile([128, HK * bc * 2], BF16, tag=f"hp2_{c}")
            hp2v = hp2[:].rearrange("p (m b two) -> p m b two", m=HK, two=2)
            nc.vector.tensor_copy(
                hp2v[:],
                php[:]
                .rearrange("p (m b2) -> p m b2", m=HK)
                .unsqueeze(3)
                .broadcast_to([128, HK, bc, 2]),
            )
        pe2 = ps_mix.tile([128, HK * nj], F32, tag="mix", name=f"pe2_{c}")
        # emit all adds first: they land consecutively in the DVE queue, so
        # interleaved copies/ctx work can't head-of-line-block the pipeline
        srcs = []
        for k in range(HK):
            sl = hproj[c][:, k * bt : (k + 1) * bt]
            if s == 0:
                srcs.append(sl)
            else:
                a = pw.tile([128, bt], BF16, tag=f"a{c}", bufs=3)
                nc.vector.tensor_add(
                    a[:].rearrange("p (b t2 two) -> p b t2 two", b=bc, two=2),
                    sl.rearrange("p (b t2 two) -> p b t2 two", b=bc, two=2),
                    hp2v[:, k].unsqueeze(2).broadcast_to([128, bc, T // 2, 2]),
                )
                srcs.append(a[:])
        for k in range(HK):
            if s == 0:
                tht = pw.tile([128, bt], BF16, tag=f"th{c}", bufs=4)
                for n in range(bt // 512):
                    nc.scalar.activation(
                        tht[:, 512 * n : 512 * (n + 1)],
                        srcs[k][:, 512 * n : 512 * (n + 1)],
                        AF.Tanh,
                    )
                th = tht[:]
            else:
                # in-place tanh on the add result (elementwise, safe): saves
                # a tile hop and SBUF traffic
                th = srcs[k]
                nc.scalar.activation(th, th, AF.Tanh)
            for j in range(nj):
                nc.tensor.matmul(
                    pe2[:, k * nj + j : k * nj + j + 1],
                    th[:, 128 * j : 128 * j + 128],
                    w_sc[:, k : k + 1],
                    start=(k == 0 and j == 0),
                    stop=(k == HK - 1 and j == nj - 1),
                    skip_group_check=True,
                )
            if interleave is not None:
                interleave(k)
        return pe2

    def softmax_a(c, pe2):
        """e-reduce, transpose, exp, row-sums, reciprocal, normalize."""
        e2 = psm.tile([128, nj], F32, tag="e2sb")
        nc.vector.reduce_sum(
            e2[:],
            pe2[:].rearrange("p (k j) -> p j k", k=HK),
            axis=mybir.AxisListType.X,
        )
        ptr = ps_tr.tile([nj, 128], F32, tag="tr")
        nc.tensor.transpose(ptr[:], e2[:], id_f[:])
        ex = psm.tile([nj, 128], F32, tag="ex")
        nc.scalar.activation(ex[:], ptr[:], AF.Exp)
        ssum = psm.tile([nj, 2], F32, tag="ssum")
        nc.vector.reduce_sum(
            ssum[:], ex[:].rearrange("p (b t) -> p b t", b=2),
            axis=mybir.AxisListType.X,
        )
        rinv = psm.tile([nj, 2], F32, tag="rinv")
        nc.vector.reciprocal(rinv[:], ssum[:])
        al = psm.tile([nj, 128], F32, tag="al")
        nc.vector.tensor_mul(
            al[:].rearrange("p (b t) -> p b t", b=2),
            ex[:].rearrange("p (b t) -> p b t", b=2),
            rinv[:].unsqueeze(2).broadcast_to([nj, 2, T]),
        )
        return al

    def softmax_b(c, al):
        """alpha back to bt-partitions, block-diag bands."""
        pac = ps_tr.tile([128, nj], F32, tag="tr")
        nc.tensor.transpose(pac[:], al[:], id_f[0:nj, 0:nj])
        adv = ad[c][:].rearrange("p (i two) -> p i two", two=2)
        for jj in range(2):
            nc.vector.tensor_copy(
                adv[64 * jj : 64 * jj + 64, :, jj], pac[64 * jj : 64 * jj + 64, :]
            )

    def ctx_and_gates(c):
        """Dense ctx MMs, ONE psum->sbuf copy, dense ctx-gates: minimizes
        the number of cross-engine sync chains (each costs ~1.3us dead)."""
        pctxT = ps_ctx.tile([128, HK * bc], F32, tag="ctxT_ps")
        for m in range(HK):
            for i in range(bc // 2):
                nc.tensor.matmul(
                    pctxT[:, m * bc + 2 * i : m * bc + 2 * i + 2],
                    enc_sb[c][:, 512 * i + 128 * m : 512 * i + 128 * m + 128],
                    ad[c][:, 2 * i : 2 * i + 2],
                    start=(i == 0),
                    stop=(i == bc // 2 - 1),
                    skip_group_check=True,
                )
        ctxc = ctxT[:, c * GWc : (c + 1) * GWc]
        nc.vector.tensor_copy(
            ctxc.rearrange("p (k b) -> p k b", k=HK),
            pctxT[:].rearrange("p (k b) -> p k b", k=HK),
        )
        pg = pgs[c]
        for mo in range(16):
            gate, k = mo // 4, mo % 4
            col = pg[:, PCOL[gate] * GWc + k * bc : PCOL[gate] * GWc + (k + 1) * bc]
            for kk in range(HK):
                nc.tensor.matmul(
                    col,
                    w_ctx[:, kk * G4 + 128 * mo : kk * G4 + 128 * mo + 128],
                    ctxc[:, kk * bc : (kk + 1) * bc],
                    start=False,
                    stop=(mo == 15 and kk == HK - 1),
                    skip_group_check=True,
                )

    def lstm_php(c):
        """Doubled-state LSTM on chunk c's columns + next-step hp psum."""
        pg = pgs[c]
        cTc = cT[:, c * GWc : (c + 1) * GWc]
        t_all = psm.tile([128, 4 * GWc], F32, tag=f"t_all{c}", bufs=2)
        nc.scalar.activation(t_all[:], pg[:, :], AF.Tanh)
        tg = t_all[:, 3 * GWc : 4 * GWc]
        x1 = psm.tile([128, GWc], F32, tag=f"m1_{c}", bufs=2)
        nc.vector.scalar_tensor_tensor(
            x1[:], t_all[:, GWc : 2 * GWc], 1.0, cTc, ALU.add, ALU.mult
        )
        x2 = psm.tile([128, GWc], F32, tag=f"m2_{c}", bufs=2)
        nc.vector.scalar_tensor_tensor(
            x2[:], t_all[:, 0:GWc], 1.0, tg, ALU.add, ALU.mult
        )
        nc.vector.scalar_tensor_tensor(
            cTc, x1[:], 0.5, x2[:], ALU.mult, ALU.add
        )
        tc_ = psm.tile([128, GWc], F32, tag=f"tc{c}", bufs=2)
        nc.scalar.activation(tc_[:], cTc, AF.Tanh, scale=0.5)
        nc.vector.scalar_tensor_tensor(
            hsv[:, :, s + 1, c * bc : (c + 1) * bc],
            t_all[:, 2 * GWc : 3 * GWc].rearrange("p (k b) -> p k b", k=HK),
            1.0,
            tc_[:].rearrange("p (k b) -> p k b", k=HK),
            ALU.add,
            ALU.mult,
        )
        if s < steps - 1:
            php = ps_mix.tile([128, GWc], F32, tag=f"php{c}", name=f"php{c}", bufs=1)
            php_holder[c] = php
            for k in range(HK):
                for m in range(HK):
                    nc.tensor.matmul(
                        php[:, m * bc : (m + 1) * bc],
                        w_h2h[:, k * H + 128 * m : k * H + 128 * m + 128],
                        hsv[:, k, s + 1, c * bc : (c + 1) * bc],
                        start=(k == 0 and m == 0),
                        stop=(k == HK - 1 and m == HK - 1),
                        skip_group_check=True,
                    )

    # -- emission: chunk pipelines offset ~half a step; chunk 1's LSTM from
    #    the PREVIOUS step (pending) lands first so its php/h2 are early --
    gates_hh(0)
    if pending is not None:
        pending()
    gates_hh(1)
    pe2_0 = attention_front(0)
    al0 = softmax_a(0, pe2_0)
    softmax_b(0, al0)
    pe2_1 = attention_front(1)
    ctx_and_gates(0)
    al1 = softmax_a(1, pe2_1)
    softmax_b(1, al1)
    ctx_and_gates(1)
    lstm_php(0)
    return lambda: lstm_php(1)


# ------------------------- host side -------------------------


def prep_inputs(encoder_output, text, w_i2h, w_h2h, b_h2h, w_score, w_ih, w_hh,
                b_ih, b_hh, w_gen, b_gen, steps=S, nchunk=NCHUNK):
    """Build per-core input maps (numpy only)."""
    bc = BCORE // nchunk
    bt = bc * T
    enc = np.asarray(encoder_output, np.float32)
    text = np.asarray(text)

    # pre-scale i,f,o gate rows (W row-blocks: i=0:512, f=512:1024, g=1024:1536,
    # o=1536:2048) by 0.5 so sigmoid(x) = 0.5*tanh(x/2)+0.5 needs one tanh
    gate_scale = np.ones((G4, 1), np.float32)
    gate_scale[0:H] = 0.5
    gate_scale[H : 2 * H] = 0.5
    gate_scale[3 * H : 4 * H] = 0.5

    w_ih_s = np.asarray(w_ih, np.float32) * gate_scale
    w_hh_s = np.asarray(w_hh, np.float32) * gate_scale
    bias_s = (np.asarray(b_ih, np.float32) + np.asarray(b_hh, np.float32)) * gate_scale[:, 0]

    wid = {}
    wid["w_i2hT"] = _tile128(np.asarray(w_i2h, np.float32).T.astype(BF))
    wid["w_h2hT"] = _tile128((0.5 * np.asarray(w_h2h, np.float32)).T.astype(BF))
    wid["w_scoreT"] = _tile128(np.asarray(w_score, np.float32).reshape(H, 1).astype(BF))
    wid["w_ctxT"] = _tile128(w_ih_s[:, :D].T.astype(BF))
    wid["w_hhT"] = _tile128((0.5 * w_hh_s).T.astype(BF))
    woh = np.zeros((128, G4), BF)  # K padded to 128 so FWL kicks in
    woh[:C] = w_ih_s[:, D:].T.astype(BF)
    woh[C] = bias_s.astype(BF)
    wid["w_ohT"] = woh
    wid["w_genT"] = _tile128((0.5 * np.asarray(w_gen, np.float32)).T.astype(BF))
    wid["b_gen"] = np.asarray(b_gen, np.float32).reshape(1, C).astype(BF)
    wid["b_h2hT"] = np.ascontiguousarray(
        np.asarray(b_h2h, np.float32).reshape(HK, 128).T
    )
    wid["id_f32"] = np.eye(128, dtype=np.float32)
    wid["ones_row"] = np.ones((1, 128), BF)

    in_maps = []
    for core in range(NCORES):
        rows = slice(core * BCORE, (core + 1) * BCORE)
        ec = enc[rows]  # [64, T, D]
        enc_sb = np.zeros((nchunk, 128, (bt // 128) * 512), BF)
        encT_sb = np.zeros((nchunk, 128, HK * bt), BF)
        for c in range(nchunk):
            flat = ec[c * bc : (c + 1) * bc].reshape(bt, D)  # b-major (b,t) rows
            enc_sb[c] = _tile128(flat.astype(BF))
            encT_sb[c] = _tile128(np.ascontiguousarray(flat.T).astype(BF))
        oh = np.zeros((128, steps * BCORE), BF)
        tx = text[rows]  # [64, S]
        for s in range(steps):
            oh[tx[:, s].astype(np.int64), s * BCORE + np.arange(BCORE)] = 1.0
        oh[C] = 1.0
        m = dict(wid)
        m["enc_sb"] = enc_sb
        m["encT_sb"] = encT_sb
        m["ohT_sb"] = oh
        in_maps.append(m)
    return in_maps


_NC_CACHE = {}


def get_nc(steps=S, nchunk=NCHUNK, n_gps_adds=0):
    key = (steps, nchunk, n_gps_adds)
    if key not in _NC_CACHE:
        _NC_CACHE[key] = build_nc(steps, nchunk, n_gps_adds)
    return _NC_CACHE[key]


def run(inputs, steps=S, nchunk=NCHUNK, n_gps_adds=0, trace=False):
    nc = get_nc(steps, nchunk, n_gps_adds)
    in_maps = prep_inputs(**inputs, steps=steps, nchunk=nchunk)
    res = run_bass_kernel_spmd(nc, in_maps, list(range(NCORES)), trace=trace)
    out = np.concatenate(
        [
            res.results[i]["probs"].reshape(steps, BCORE, C).transpose(1, 0, 2)
            for i in range(NCORES)
        ],
        axis=0,
    )
    return out.astype(np.float32), res


def kernel(**inputs):
    out, _ = run(inputs)
    return out



# revision 7
# speedup vs baseline: 1.0306x; 1.0306x over previous
"""Trainium2 Bass kernel: attention-LSTM decoder (nn_Attention_74698071212133).

Sharding: data-parallel over batch across 8 NeuronCores (64 rows each), weights
replicated.  Each core splits its 64 rows into 2 chunks of 32 that run as
software-pipelined per-chunk pipelines, offset roughly half a step; chunk 1's
LSTM emission is deferred into the next step so its tail overlaps the next
step's chunk-0 attention (the whole recurrence splits cleanly by batch).

Per-core, per chunk, per step (bc=32, T=64, H=512, C=38):
  a  = H_projT + bcast_t(hpT)          (DVE bf16 2x, all adds emitted first)
  th = tanh(a)                         (ACT - the per-step bottleneck engine)
  e  = w_score . th -> psum            (PE, th stationary, one accum group)
  softmax via PE transposes            (split A/B emission)
  ctxT = enc.T @ alpha-blockdiag       (PE [d,b] direct; dense block, one
                                        psum->sbuf copy per chunk)
  gates into one psum [128,(gate,k,b)] (i/f/o rows pre-halved -> single tanh)
  LSTM on doubled states h2=2h, c2=2c  (4 scalar_tensor_tensor ops; consumer
                                        weights w_hh/w_h2h/w_gen pre-halved,
                                        tanh(c) via ACT scale=0.5)
  hp(next) = w_h2h @ h2 -> psum        (PE, per chunk)
h states live in the hs_all ring (k-major [k][s][b]: no copies, no WAR);
probs = one batched matmul over all 26 steps after the scan.
"""

import sys

sys.path.insert(0, "/opt/trn_rl_repo")

import numpy as np
import ml_dtypes

import concourse.bass as bass
import concourse.mybir as mybir
import concourse.tile as tile
from concourse import bacc
from concourse.bass_utils import run_bass_kernel_spmd

BF = ml_dtypes.bfloat16
F32 = mybir.dt.float32
BF16 = mybir.dt.bfloat16
AF = mybir.ActivationFunctionType
ALU = mybir.AluOpType

# Problem constants
B, T, D, H, C, S = 512, 64, 512, 512, 38, 26
NCORES = 8
BCORE = B // NCORES  # 64
NCHUNK = 2
G4 = 4 * H  # 2048
HK = H // 128  # 4 h-tiles
GW = HK * BCORE  # 256: one h-state slot (k-major, then b)
PCOL = (0, 1, 3, 2)  # W gate order i,f,g,o -> psum col block [i | f | o | g]


def _tile128(a):
    """[R, N] with R = r*128 -> [128, r*N] col-block layout (block k = rows 128k..)."""
    r = a.shape[0] // 128
    return np.ascontiguousarray(
        a.reshape(r, 128, a.shape[1]).transpose(1, 0, 2).reshape(128, -1)
    )


def build_nc(steps=S, nchunk=NCHUNK, n_gps_adds=0):
    bc = BCORE // nchunk  # batch per chunk
    bt = bc * T  # flattened (b, t) per chunk, b-major
    nbt = bt // 128  # 128-row bt tiles per chunk

    nc = bacc.Bacc()
    dp = nc.declare_dram_parameter
    # Per-core tensors (pre-tiled on host into [128, cols] SBUF images)
    d_enc = dp("enc_sb", [nchunk, 128, nbt * 512], BF16, isOutput=False)
    d_encT = dp("encT_sb", [nchunk, 128, HK * bt], BF16, isOutput=False)
    d_oh = dp("ohT_sb", [128, steps * BCORE], BF16, isOutput=False)
    # Replicated weights
    d_wi2h = dp("w_i2hT", [128, HK * H], BF16, isOutput=False)
    d_wh2h = dp("w_h2hT", [128, HK * H], BF16, isOutput=False)
    d_wsc = dp("w_scoreT", [128, HK], BF16, isOutput=False)
    d_wctx = dp("w_ctxT", [128, HK * G4], BF16, isOutput=False)
    d_whh = dp("w_hhT", [128, HK * G4], BF16, isOutput=False)
    d_woh = dp("w_ohT", [128, G4], BF16, isOutput=False)
    d_wgen = dp("w_genT", [128, HK * C], BF16, isOutput=False)
    d_bgen = dp("b_gen", [1, C], BF16, isOutput=False)
    d_bh2h = dp("b_h2hT", [128, HK], F32, isOutput=False)
    d_idf = dp("id_f32", [128, 128], F32, isOutput=False)
    d_ones = dp("ones_row", [1, 128], BF16, isOutput=False)
    d_out = dp("probs", [steps * BCORE, C], F32, isOutput=True)  # s-major rows

    with tile.TileContext(nc) as tc:
        with (
            tc.tile_pool(name="consts", bufs=1) as pc,
            tc.tile_pool(name="persist", bufs=1) as pp,
        ):
            # ---- load constants ----
            def cload(dram, shape, dt):
                t_ = pc.tile(list(shape), dt, name=dram.tensor.name + "_sb")
                nc.sync.dma_start(t_[:], dram)
                return t_

            # only what init needs loads first; the 4MB+ of gate weights
            # (w_ctx/w_hh/...) load AFTER encT so they don't starve H_proj
            w_i2h = cload(d_wi2h[:], [128, HK * H], BF16)
            w_sc = cload(d_wsc[:], [128, HK], BF16)
            b_gen = cload(d_bgen[:], [1, C], BF16)
            b_h2h = cload(d_bh2h[:], [128, HK], F32)
            id_f = cload(d_idf[:], [128, 128], F32)
            ones = cload(d_ones[:], [1, 128], BF16)

            # ---- persistent state ----
            # hs_all: ring of h states, slot s+1 = h after step s (slot 0 = 0)
            # hs_all layout: k-major [k][s][b]; slot s+1 = h after step s
            hs_all = pp.tile([128, HK * (steps + 1) * BCORE], BF16, tag="hs_all")
            hsv = hs_all[:].rearrange(
                "p (k s b) -> p k s b", k=HK, s=steps + 1
            )
            cT = pp.tile([128, GW], F32, tag="cT")
            ctxT = pp.tile([128, GW], BF16, tag="ctxT")
            for k in range(HK):
                nc.vector.memset(hsv[:, k, 0, :], 0.0)
            nc.vector.memset(cT[:], 0.0)

            enc_sb, hproj, ad = [], [], []
            for c in range(nchunk):
                e_ = pp.tile([128, nbt * 512], BF16, tag=f"enc{c}")
                enc_sb.append(e_)
                hproj.append(
                    pp.tile([128, HK * bt], BF16, tag=f"hproj{c}", name=f"hproj{c}")
                )
                a_ = pp.tile([128, bc], BF16, tag=f"ad{c}", name=f"ad{c}")
                nc.vector.memset(a_[:], 0.0)
                ad.append(a_)

            # ---- init: H_projT = w_i2h @ encT + b_h2h ----
            with (
                tc.tile_pool(name="encT", bufs=1) as pet,
                tc.tile_pool(name="initps", bufs=4, space="PSUM") as pips,
            ):
                for c in range(nchunk):
                    et = pet.tile([128, HK * bt], BF16, tag=f"encT{c}", name=f"encT{c}")
                    # column-sliced loads: slice n carries ALL k-blocks for
                    # bt-range n, so the n-th group of MMs starts after 1/nq
                    # of the load instead of waiting for the whole chunk
                    nq = bt // 512
                    etv = et[:].rearrange("p (k c2) -> p k c2", k=HK)
                    dv = d_encT[c].rearrange("p (k c2) -> p k c2", k=HK)
                    for n in range(nq):
                        nc.sync.dma_start(
                            etv[:, :, 512 * n : 512 * (n + 1)],
                            dv[:, :, 512 * n : 512 * (n + 1)],
                        )
                    for n in range(nq):
                        for m in range(HK):
                            ps = pips.tile([128, 512], F32, tag="initp")
                            for k in range(HK):
                                nc.tensor.matmul(
                                    ps[:],
                                    w_i2h[:, k * H + 128 * m : k * H + 128 * m + 128],
                                    et[:, k * bt + 512 * n : k * bt + 512 * n + 512],
                                    start=(k == 0),
                                    stop=(k == HK - 1),
                                )
                            nc.scalar.activation(
                                hproj[c][:, m * bt + 512 * n : m * bt + 512 * n + 512],
                                ps[:],
                                AF.Identity,
                                bias=b_h2h[:, m : m + 1],
                            )

            # deferred loads, ordered by first use in step 0:
            # enc c0 (ctx@~17us), gate weights (tanh_all@~20), w_ctx
            # (ctx-gates), w_h2h (php@~24), enc c1 (ctx c1@~25), w_gen (probs)
            for q in range(4):
                w = nbt * 512 // 4
                nc.sync.dma_start(
                    enc_sb[0][:, q * w : (q + 1) * w],
                    d_enc[0, :, q * w : (q + 1) * w],
                )
            w_hh = cload(d_whh[:], [128, HK * G4], BF16)
            w_oh = cload(d_woh[:], [128, G4], BF16)
            ohT = cload(d_oh[:], [128, steps * BCORE], BF16)
            w_ctx = cload(d_wctx[:], [128, HK * G4], BF16)
            w_h2h = cload(d_wh2h[:], [128, HK * H], BF16)
            for c in range(1, nchunk):
                for q in range(4):
                    w = nbt * 512 // 4
                    nc.sync.dma_start(
                        enc_sb[c][:, q * w : (q + 1) * w],
                        d_enc[c, :, q * w : (q + 1) * w],
                    )
            w_gen = cload(d_wgen[:], [128, HK * C], BF16)

            # ---- decode steps ----
            with (
                tc.tile_pool(name="work", bufs=6) as pw,
                tc.tile_pool(name="small", bufs=4) as psm,
                tc.tile_pool(name="ps_mix", bufs=2, space="PSUM") as ps_mix,
                tc.tile_pool(name="ps_tr", bufs=1, space="PSUM") as ps_tr,
                tc.tile_pool(name="ps_ctx", bufs=1, space="PSUM") as ps_ctx,
                tc.tile_pool(name="ps_g", bufs=1, space="PSUM") as ps_g,
            ):
                ntile = (steps * BCORE) // 128  # 13

                def emit_probs_tile(t_):
                    pp_ = ps_mix.tile([128, C], F32, tag="mix", name="pp_")
                    kstride = (steps + 1) * BCORE
                    for k in range(HK):
                        base = k * kstride + (2 * t_ + 1) * BCORE
                        nc.tensor.matmul(
                            pp_[:],
                            hs_all[:, base : base + 128],
                            w_gen[:, k * C : (k + 1) * C],
                            start=(k == 0), stop=False, skip_group_check=True,
                        )
                    nc.tensor.matmul(
                        pp_[:], ones[0:1, :], b_gen[:],
                        start=False, stop=True, skip_group_check=True,
                    )
                    po = psm.tile([128, C], F32, tag="po")
                    nc.vector.tensor_copy(po[:], pp_[:])
                    nc.sync.dma_start(d_out[t_ * 128 : (t_ + 1) * 128, :], po[:])

                php_holder = [None] * NCHUNK
                pending = None
                for s in range(steps):
                    pending = step_body(
                        nc, tc, s, steps, nchunk, bc, bt,
                        pw, psm, ps_mix, ps_tr, ps_ctx, ps_g,
                        enc_sb, hproj, hsv, cT, ctxT, ad,
                        w_h2h, w_sc, w_ctx, w_hh, w_oh,
                        ohT, id_f, php_holder, n_gps_adds, pending,
                    )
                    # probs tile t covers h-slots (2t+1, 2t+2): fully written
                    # once step 2t+2's deferred c1-LSTM has been emitted
                    if s >= 2 and s % 2 == 0:
                        emit_probs_tile((s - 2) // 2)
                pending()  # flush chunk 1's final LSTM

                # ---- final probs tile (12 of 13 were emitted in-scan) ----
                emit_probs_tile(ntile - 1)
    if not nc.is_finalized():
        nc.finalize()
    return nc


def step_body(
    nc, tc, s, steps, nchunk, bc, bt,
    pw, psm, ps_mix, ps_tr, ps_ctx, ps_g,
    enc_sb, hproj, hsv, cT, ctxT, ad,
    w_h2h, w_sc, w_ctx, w_hh, w_oh,
    ohT, id_f, php_holder, n_gps_adds, pending,
):
    nj = bt // 128
    BW = nchunk * bc
    GWc = HK * bc  # per-chunk gates col width

    # -- per-chunk gates psum + hh/oh contributions (PE work during attention)
    pgs = {}

    def gates_hh(c):
        pg = ps_g.tile([128, 4 * GWc], F32, tag=f"g{c}", name=f"pg{c}")
        pgs[c] = pg
        ohsl = ohT[:, s * BW + c * bc : s * BW + (c + 1) * bc]
        for m in range(16):
            gate, k = m // 4, m % 4
            col = pg[:, PCOL[gate] * GWc + k * bc : PCOL[gate] * GWc + (k + 1) * bc]
            for kk in range(HK):
                nc.tensor.matmul(
                    col,
                    w_hh[:, kk * G4 + 128 * m : kk * G4 + 128 * m + 128],
                    hsv[:, kk, s, c * bc : (c + 1) * bc],
                    start=(m == 0 and kk == 0),
                    stop=False,
                    skip_group_check=True,
                )
            nc.tensor.matmul(
                col, w_oh[:, 128 * m : 128 * m + 128], ohsl,
                start=False, stop=False, skip_group_check=True,
            )

    def attention_front(c, interleave=None):
        """hp2 + adds + tanhs + score MMs for chunk c; optionally interleave
        other-chunk ctx work after each k-block (fills PE tanh-wait gaps)."""
        if s > 0:
            php = php_holder[c]
            hp2 = psm.tile([128, HK * bc * 2], BF16, tag=f"hp2_{c}")
            hp2v = hp2[:].rearrange("p (m b two) -> p m b two", m=HK, two=2)
            nc.vector.tensor_copy(
                hp2v[:],
                php[:]
                .rearrange("p (m b2) -> p m b2", m=HK)
                .unsqueeze(3)
                .broadcast_to([128, HK, bc, 2]),
            )
        pe2 = ps_mix.tile([128, HK * nj], F32, tag="mix", name=f"pe2_{c}")
        # emit all adds first: they land consecutively in the DVE queue, so
        # interleaved copies/ctx work can't head-of-line-block the pipeline
        srcs = []
        for k in range(HK):
            sl = hproj[c][:, k * bt : (k + 1) * bt]
            if s == 0:
                srcs.append(sl)
            else:
                a = pw.tile([128, bt], BF16, tag=f"a{c}", bufs=3)
                nc.vector.tensor_add(
                    a[:].rearrange("p (b t2 two) -> p b t2 two", b=bc, two=2),
                    sl.rearrange("p (b t2 two) -> p b t2 two", b=bc, two=2),
                    hp2v[:, k].unsqueeze(2).broadcast_to([128, bc, T // 2, 2]),
                )
                srcs.append(a[:])
        for k in range(HK):
            if s == 0:
                tht = pw.tile([128, bt], BF16, tag=f"th{c}", bufs=4)
                for n in range(bt // 512):
                    nc.scalar.activation(
                        tht[:, 512 * n : 512 * (n + 1)],
                        srcs[k][:, 512 * n : 512 * (n + 1)],
                        AF.Tanh,
                    )
                th = tht[:]
            else:
                # in-place tanh on the add result (elementwise, safe): saves
                # a tile hop and SBUF traffic
                th = srcs[k]
                nc.scalar.activation(th, th, AF.Tanh)
            for j in range(nj):
                nc.tensor.matmul(
                    pe2[:, k * nj + j : k * nj + j + 1],
                    th[:, 128 * j : 128 * j + 128],
                    w_sc[:, k : k + 1],
                    start=(k == 0 and j == 0),
                    stop=(k == HK - 1 and j == nj - 1),
                    skip_group_check=True,
                )
            if interleave is not None:
                interleave(k)
        return pe2

    def softmax_a(c, pe2):
        """e-reduce, transpose, exp, row-sums, reciprocal, normalize."""
        e2 = psm.tile([128, nj], F32, tag="e2sb")
        nc.vector.reduce_sum(
            e2[:],
            pe2[:].rearrange("p (k j) -> p j k", k=HK),
            axis=mybir.AxisListType.X,
        )
        ptr = ps_tr.tile([nj, 128], F32, tag="tr")
        nc.tensor.transpose(ptr[:], e2[:], id_f[:])
        ex = psm.tile([nj, 128], F32, tag="ex")
        nc.scalar.activation(ex[:], ptr[:], AF.Exp)
        ssum = psm.tile([nj, 2], F32, tag="ssum")
        nc.vector.reduce_sum(
            ssum[:], ex[:].rearrange("p (b t) -> p b t", b=2),
            axis=mybir.AxisListType.X,
        )
        rinv = psm.tile([nj, 2], F32, tag="rinv")
        nc.vector.reciprocal(rinv[:], ssum[:])
        al = psm.tile([nj, 128], F32, tag="al")
        nc.vector.tensor_mul(
            al[:].rearrange("p (b t) -> p b t", b=2),
            ex[:].rearrange("p (b t) -> p b t", b=2),
            rinv[:].unsqueeze(2).broadcast_to([nj, 2, T]),
        )
        return al

    def softmax_b(c, al):
        """alpha back to bt-partitions, block-diag bands."""
        pac = ps_tr.tile([128, nj], F32, tag="tr")
        nc.tensor.transpose(pac[:], al[:], id_f[0:nj, 0:nj])
        adv = ad[c][:].rearrange("p (i two) -> p i two", two=2)
        for jj in range(2):
            nc.vector.tensor_copy(
                adv[64 * jj : 64 * jj + 64, :, jj], pac[64 * jj : 64 * jj + 64, :]
            )

    def ctx_and_gates(c):
        """Dense ctx MMs, ONE psum->sbuf copy, dense ctx-gates: minimizes
        the number of cross-engine sync chains (each costs ~1.3us dead)."""
        pctxT = ps_ctx.tile([128, HK * bc], F32, tag="ctxT_ps", name="pctxT")
        for m in range(HK):
            for i in range(bc // 2):
                nc.tensor.matmul(
                    pctxT[:, m * bc + 2 * i : m * bc + 2 * i + 2],
                    enc_sb[c][:, 512 * i + 128 * m : 512 * i + 128 * m + 128],
                    ad[c][:, 2 * i : 2 * i + 2],
                    start=(i == 0),
                    stop=(i == bc // 2 - 1),
                    skip_group_check=True,
                )
        ctxc = ctxT[:, c * GWc : (c + 1) * GWc]
        nc.vector.tensor_copy(
            ctxc.rearrange("p (k b) -> p k b", k=HK),
            pctxT[:].rearrange("p (k b) -> p k b", k=HK),
        )
        pg = pgs[c]
        for mo in range(16):
            gate, k = mo // 4, mo % 4
            col = pg[:, PCOL[gate] * GWc + k * bc : PCOL[gate] * GWc + (k + 1) * bc]
            for kk in range(HK):
                nc.tensor.matmul(
                    col,
                    w_ctx[:, kk * G4 + 128 * mo : kk * G4 + 128 * mo + 128],
                    ctxc[:, kk * bc : (kk + 1) * bc],
                    start=False,
                    stop=(mo == 15 and kk == HK - 1),
                    skip_group_check=True,
                )

    def lstm_act(c):
        """Gate tanh: ACT reads the pg psum group right after its stop MM,
        landing the gates in SBUF so the psum frees early and the deferred
        tail only depends on SBUF data."""
        pg = pgs[c]
        t_all = psm.tile([128, 4 * GWc], F32, tag=f"t_all{c}", bufs=2)
        nc.scalar.activation(t_all[:], pg[:, :], AF.Tanh)
        return t_all

    def lstm_tail(c, t_all):
        """Doubled-state LSTM on chunk c's columns + next-step hp psum."""
        cTc = cT[:, c * GWc : (c + 1) * GWc]
        tg = t_all[:, 3 * GWc : 4 * GWc]
        x1 = psm.tile([128, GWc], F32, tag=f"m1_{c}", bufs=2)
        nc.vector.scalar_tensor_tensor(
            x1[:], t_all[:, GWc : 2 * GWc], 1.0, cTc, ALU.add, ALU.mult
        )
        x2 = psm.tile([128, GWc], F32, tag=f"m2_{c}", bufs=2)
        nc.vector.scalar_tensor_tensor(
            x2[:], t_all[:, 0:GWc], 1.0, tg, ALU.add, ALU.mult
        )
        nc.vector.scalar_tensor_tensor(
            cTc, x1[:], 0.5, x2[:], ALU.mult, ALU.add
        )
        tc_ = psm.tile([128, GWc], F32, tag=f"tc{c}", bufs=2)
        nc.scalar.activation(tc_[:], cTc, AF.Tanh, scale=0.5)
        nc.vector.scalar_tensor_tensor(
            hsv[:, :, s + 1, c * bc : (c + 1) * bc],
            t_all[:, 2 * GWc : 3 * GWc].rearrange("p (k b) -> p k b", k=HK),
            1.0,
            tc_[:].rearrange("p (k b) -> p k b", k=HK),
            ALU.add,
            ALU.mult,
        )
        if s < steps - 1:
            php = ps_mix.tile([128, GWc], F32, tag=f"php{c}", name=f"php{c}", bufs=1)
            php_holder[c] = php
            for k in range(HK):
                for m in range(HK):
                    nc.tensor.matmul(
                        php[:, m * bc : (m + 1) * bc],
                        w_h2h[:, k * H + 128 * m : k * H + 128 * m + 128],
                        hsv[:, k, s + 1, c * bc : (c + 1) * bc],
                        start=(k == 0 and m == 0),
                        stop=(k == HK - 1 and m == HK - 1),
                        skip_group_check=True,
                    )

    # -- emission: dataflow order. Chunk 1's LSTM tail from the PREVIOUS
    #    step (pending) lands first so its h/php are early; each chunk's
    #    gate-tanh (lstm_act) is emitted right after its psum group stops
    #    so the ACT read doesn't pick up a conservative late PE wait --
    if pending is not None:
        pending()
    gates_hh(0)
    gates_hh(1)
    pe2_0 = attention_front(0)
    al0 = softmax_a(0, pe2_0)
    softmax_b(0, al0)
    ctx_and_gates(0)
    ta0 = lstm_act(0)
    pe2_1 = attention_front(1)
    al1 = softmax_a(1, pe2_1)
    softmax_b(1, al1)
    ctx_and_gates(1)
    ta1 = lstm_act(1)
    lstm_tail(0, ta0)
    return lambda: lstm_tail(1, ta1)


# ------------------------- host side -------------------------


def prep_inputs(encoder_output, text, w_i2h, w_h2h, b_h2h, w_score, w_ih, w_hh,
                b_ih, b_hh, w_gen, b_gen, steps=S, nchunk=NCHUNK):
    """Build per-core input maps (numpy only)."""
    bc = BCORE // nchunk
    bt = bc * T
    enc = np.asarray(encoder_output, np.float32)
    text = np.asarray(text)

    # pre-scale i,f,o gate rows (W row-blocks: i=0:512, f=512:1024, g=1024:1536,
    # o=1536:2048) by 0.5 so sigmoid(x) = 0.5*tanh(x/2)+0.5 needs one tanh
    gate_scale = np.ones((G4, 1), np.float32)
    gate_scale[0:H] = 0.5
    gate_scale[H : 2 * H] = 0.5
    gate_scale[3 * H : 4 * H] = 0.5

    w_ih_s = np.asarray(w_ih, np.float32) * gate_scale
    w_hh_s = np.asarray(w_hh, np.float32) * gate_scale
    bias_s = (np.asarray(b_ih, np.float32) + np.asarray(b_hh, np.float32)) * gate_scale[:, 0]

    wid = {}
    wid["w_i2hT"] = _tile128(np.asarray(w_i2h, np.float32).T.astype(BF))
    wid["w_h2hT"] = _tile128((0.5 * np.asarray(w_h2h, np.float32)).T.astype(BF))
    wid["w_scoreT"] = _tile128(np.asarray(w_score, np.float32).reshape(H, 1).astype(BF))
    wid["w_ctxT"] = _tile128(w_ih_s[:, :D].T.astype(BF))
    wid["w_hhT"] = _tile128((0.5 * w_hh_s).T.astype(BF))
    woh = np.zeros((128, G4), BF)  # K padded to 128 so FWL kicks in
    woh[:C] = w_ih_s[:, D:].T.astype(BF)
    woh[C] = bias_s.astype(BF)
    wid["w_ohT"] = woh
    wid["w_genT"] = _tile128((0.5 * np.asarray(w_gen, np.float32)).T.astype(BF))
    wid["b_gen"] = np.asarray(b_gen, np.float32).reshape(1, C).astype(BF)
    wid["b_h2hT"] = np.ascontiguousarray(
        np.asarray(b_h2h, np.float32).reshape(HK, 128).T
    )
    wid["id_f32"] = np.eye(128, dtype=np.float32)
    wid["ones_row"] = np.ones((1, 128), BF)

    in_maps = []
    for core in range(NCORES):
        rows = slice(core * BCORE, (core + 1) * BCORE)
        ec = enc[rows]  # [64, T, D]
        enc_sb = np.zeros((nchunk, 128, (bt // 128) * 512), BF)
        encT_sb = np.zeros((nchunk, 128, HK * bt), BF)
        for c in range(nchunk):
            flat = ec[c * bc : (c + 1) * bc].reshape(bt, D)  # b-major (b,t) rows
            enc_sb[c] = _tile128(flat.astype(BF))
            encT_sb[c] = _tile128(np.ascontiguousarray(flat.T).astype(BF))
        oh = np.zeros((128, steps * BCORE), BF)
        tx = text[rows]  # [64, S]
        for s in range(steps):
            oh[tx[:, s].astype(np.int64), s * BCORE + np.arange(BCORE)] = 1.0
        oh[C] = 1.0
        m = dict(wid)
        m["enc_sb"] = enc_sb
        m["encT_sb"] = encT_sb
        m["ohT_sb"] = oh
        in_maps.append(m)
    return in_maps


_NC_CACHE = {}


def get_nc(steps=S, nchunk=NCHUNK, n_gps_adds=0):
    key = (steps, nchunk, n_gps_adds)
    if key not in _NC_CACHE:
        _NC_CACHE[key] = build_nc(steps, nchunk, n_gps_adds)
    return _NC_CACHE[key]


def run(inputs, steps=S, nchunk=NCHUNK, n_gps_adds=0, trace=False):
    nc = get_nc(steps, nchunk, n_gps_adds)
    in_maps = prep_inputs(**inputs, steps=steps, nchunk=nchunk)
    res = run_bass_kernel_spmd(nc, in_maps, list(range(NCORES)), trace=trace)
    out = np.concatenate(
        [
            res.results[i]["probs"].reshape(steps, BCORE, C).transpose(1, 0, 2)
            for i in range(NCORES)
        ],
        axis=0,
    )
    return out.astype(np.float32), res


def kernel(**inputs):
    out, _ = run(inputs)
    return out

